# revision 6
# baseline (speedup 1.0000x reference)
"""Trainium2 Bass kernel for nn_AccumulatingModule (histogram_binning).

Problem: out = score_matrix.at[qt, p, ol1, ol2].add(at1*at2) — a scatter-add of
BATCH*PAIR outer-product contributions into a [65, 90, 151, 151] fp32 histogram.

Strategy (8 NeuronCores, SPMD):
  * The memory roofline is streaming score_matrix (533 MB) in + out once.
  * Shard the (qt, pair) space: each qt's 90 pairs are split into two
    45-pair "half sections" (pairs with first-box-index i in {0..4} and
    {5..9}).  130 half-sections + 6 dummies = 17 per core.
  * A box permutation trick keeps the compiled kernel identical across
    cores (SPMD): every section computes the FIXED pattern pairs
    {(i,j): i in 0..4, j != i}; the host permutes the 10 box columns of
    obj_label/attention per section so the pattern maps onto the actual
    pairs, and orders score rows to match the kernel's slot order.
  * Per section: batch rows with that qt (max 154, padded to 2x128 chunks).
    W[b,k,:] = attention[b,k] * onehot(label[b,k]) built on VectorE
    (tensor_scalar is_equal*mult against an iota row), stored bf16
    (one-hot side exact, at rounded once -> ~2^-9 relative error on the
    sparse delta only).
  * delta[pair(i,j)] = W_j^T @ W_i on TensorE, accumulated in PSUM over the
    two 128-row chunks.  o1=151 splits into a 128-row main piece and a
    23-row tail; tails of 4 groups share one PSUM bank at partition offsets
    0/32/64/96 via matmul col-tiling so the fp32 adds stay cheap.
  * out = psum + score tile on VectorE, DMA'd straight back out.
"""

import numpy as np

NUM_QT, NUM_OT, PAIR = 65, 151, 90
BOX = 10
OT = NUM_OT
ROWLEN = OT * OT  # 22801
SECP = 45  # pairs per (half) section
NSEC = 17  # sections per core
NCORES = 8
ROWS_PER_SEC = 256  # padded batch rows per section (2 chunks of 128)
TAIL_BANDS = 4  # tail groups sharing one PSUM bank


def _pattern_groups():
    """Groups of pattern pairs sharing stationary j with consecutive moving i.

    Pattern pairs: (i, j) with i in 0..4, j in 0..9, j != i.
    Returns list of (j, istart, gsize) with gsize <= 3.
    """
    groups = []
    for j in range(BOX):
        ilist = [i for i in range(5) if i != j]
        runs = []
        cur = [ilist[0]]
        for i in ilist[1:]:
            if i == cur[-1] + 1:
                cur.append(i)
            else:
                runs.append(cur)
                cur = [i]
        runs.append(cur)
        for run in runs:
            for cs in range(0, len(run), 3):
                chunk = run[cs : cs + 3]
                groups.append((j, chunk[0], len(chunk)))
    return groups


GROUPS = _pattern_groups()
SLOTS = [(i, j) for (j, i0, g) in GROUPS for i in range(i0, i0 + g)]
assert len(SLOTS) == SECP and len(set(SLOTS)) == SECP


def build_nc(nsec=NSEC):
    import concourse.bacc as bacc
    import concourse.tile as tile
    from concourse import mybir
    from contextlib import ExitStack

    f32 = mybir.dt.float32
    bf16 = mybir.dt.bfloat16

    nc = bacc.Bacc(None, target_bir_lowering=False)
    score_in = nc.dram_tensor(
        "score_in", [nsec * SECP, ROWLEN], f32, kind="ExternalInput"
    )
    meta = nc.dram_tensor(
        "meta", [nsec * ROWS_PER_SEC, 2 * BOX], f32, kind="ExternalInput"
    )
    iota = nc.dram_tensor("iota", [128, OT], f32, kind="ExternalInput")
    score_out = nc.dram_tensor(
        "score_out", [nsec * SECP, ROWLEN], f32, kind="ExternalOutput"
    )

    with tile.TileContext(nc) as tc, ExitStack() as ctx:
        const_pool = ctx.enter_context(tc.tile_pool(name="const", bufs=1))
        meta_pool = ctx.enter_context(tc.tile_pool(name="meta", bufs=4))
        w_pool = ctx.enter_context(tc.tile_pool(name="w", bufs=4))
        sin_pool = ctx.enter_context(tc.tile_pool(name="sin", bufs=6))
        sout_pool = ctx.enter_context(tc.tile_pool(name="sout", bufs=6))
        tin_pool = ctx.enter_context(tc.tile_pool(name="tin", bufs=2))
        tout_pool = ctx.enter_context(tc.tile_pool(name="tout", bufs=2))
        pm_pool = ctx.enter_context(tc.tile_pool(name="pm", bufs=4, space="PSUM"))
        pt_pool = ctx.enter_context(tc.tile_pool(name="pt", bufs=2, space="PSUM"))

        iota_t = const_pool.tile([128, OT], f32)
        nc.sync.dma_start(iota_t[:], iota[:])
        zeros_t = const_pool.tile([128, 512], f32)
        nc.vector.memset(zeros_t[:], 0.0)

        for s in range(nsec):
            mt = []
            for c in range(2):
                t = meta_pool.tile([128, 2 * BOX], f32)
                nc.sync.dma_start(
                    t[:],
                    meta[s * ROWS_PER_SEC + c * 128 : s * ROWS_PER_SEC + (c + 1) * 128, :],
                )
                mt.append(t)
            W = []
            for c in range(2):
                w = w_pool.tile([128, BOX, OT], bf16)
                for k in range(BOX):
                    nc.vector.tensor_scalar(
                        w[:, k, :],
                        iota_t[:],
                        mt[c][:, k : k + 1],
                        mt[c][:, BOX + k : BOX + k + 1],
                        mybir.AluOpType.is_equal,
                        mybir.AluOpType.mult,
                    )
                W.append(w)

            goff = np.concatenate([[0], np.cumsum([g for (_, _, g) in GROUPS])])
            for c0 in range(0, len(GROUPS), TAIL_BANDS):
                cluster = list(enumerate(GROUPS))[c0 : c0 + TAIL_BANDS]
                ptt = pt_pool.tile([128, 512], f32)
                stt = tin_pool.tile([128, 512], f32)
                ott = tout_pool.tile([128, 512], f32)
                nc.scalar.copy(ptt[:], zeros_t[:])
                nc.scalar.copy(stt[:], zeros_t[:])
                tail_hi = 32 * (len(cluster) - 1) + 23
                for gi, (j, i0, g) in cluster:
                    band = gi - c0
                    t0 = int(goff[gi])
                    rows_in = score_in[
                        s * SECP + t0 : s * SECP + t0 + g, :
                    ].rearrange("g (a b) -> a g b", b=OT)

                    # ---- main piece: o1 in [0, 128) ----
                    sm = sin_pool.tile([128, g, OT], f32)
                    nc.sync.dma_start(sm[:], rows_in[0:128])
                    psm = pm_pool.tile([128, g, OT], f32)
                    for c in range(2):
                        nc.tensor.matmul(
                            psm[:],
                            W[c][:, j, 0:128],
                            W[c][:, i0 : i0 + g, :],
                            start=(c == 0),
                            stop=(c == 1),
                        )
                    om = sout_pool.tile([128, g, OT], f32)
                    nc.vector.tensor_add(om[:], psm[:], sm[:])
                    rows_out = score_out[
                        s * SECP + t0 : s * SECP + t0 + g, :
                    ].rearrange("g (a b) -> a g b", b=OT)
                    nc.scalar.dma_start(rows_out[0:128], om[:])

                    # ---- tail: o1 in [128,151) -> band of shared PSUM bank ----
                    pb = band * 32
                    nc.sync.dma_start(
                        stt[pb : pb + 23, 0 : g * OT].rearrange(
                            "p (g b) -> p g b", b=OT
                        ),
                        rows_in[128:OT],
                    )
                    for c in range(2):
                        nc.tensor.matmul(
                            ptt[pb : pb + 23, 0 : g * OT],
                            W[c][:, j, 128:OT],
                            W[c][:, i0 : i0 + g, :],
                            start=False,
                            stop=(c == 1),
                            tile_position=(0, pb),
                            skip_group_check=True,
                        )
                nc.vector.tensor_add(
                    ott[0:tail_hi, :], ptt[0:tail_hi, :], stt[0:tail_hi, :]
                )
                for gi, (j, i0, g) in cluster:
                    band = gi - c0
                    t0 = int(goff[gi])
                    pb = band * 32
                    rows_out = score_out[
                        s * SECP + t0 : s * SECP + t0 + g, :
                    ].rearrange("g (a b) -> a g b", b=OT)
                    nc.scalar.dma_start(
                        rows_out[128:OT],
                        ott[pb : pb + 23, 0 : g * OT].rearrange(
                            "p (g b) -> p g b", b=OT
                        ),
                    )
    return nc


# ---------------------------------------------------------------------------
# host-side routing
# ---------------------------------------------------------------------------


def _sections():
    secs = [(q, h) for q in range(NUM_QT) for h in (0, 1)]
    secs += [None] * (NCORES * NSEC - len(secs))
    return secs


def _route(obj_label, qus_type, attention, score_matrix):
    score2d = np.ascontiguousarray(score_matrix).reshape(NUM_QT * PAIR, ROWLEN)
    order = np.argsort(qus_type, kind="stable")
    counts = np.bincount(qus_type, minlength=NUM_QT)
    starts = np.concatenate([[0], np.cumsum(counts)])
    secs = _sections()

    iota_arr = np.tile(np.arange(OT, dtype=np.float32), (128, 1))
    in_maps = []
    core_rows = []  # per core: [NSEC*SECP] index into score2d or -1
    for core in range(NCORES):
        sc_rows = np.full(NSEC * SECP, -1, np.int64)
        meta = np.zeros((NSEC * ROWS_PER_SEC, 2 * BOX), np.float32)
        for sl in range(NSEC):
            sec = secs[core * NSEC + sl]
            if sec is None:
                continue
            q, h = sec
            perm = np.array([(x + 5) % 10 if h else x for x in range(BOX)])
            rows = order[starts[q] : starts[q + 1]]
            B = len(rows)
            assert B <= ROWS_PER_SEC, f"group {q} has {B} rows > {ROWS_PER_SEC}"
            meta[sl * ROWS_PER_SEC : sl * ROWS_PER_SEC + B, 0:BOX] = obj_label[rows][
                :, perm
            ].astype(np.float32)
            meta[sl * ROWS_PER_SEC : sl * ROWS_PER_SEC + B, BOX:] = attention[rows][
                :, perm
            ]
            for t, (i, j) in enumerate(SLOTS):
                I, J = perm[i], perm[j]
                p = 9 * I + (J if J < I else J - 1)
                sc_rows[sl * SECP + t] = q * PAIR + p
        score_in = score2d[np.maximum(sc_rows, 0)]
        in_maps.append(
            {
                "score_in": np.ascontiguousarray(score_in, np.float32),
                "meta": meta,
                "iota": iota_arr,
            }
        )
        core_rows.append(sc_rows)
    return in_maps, core_rows


def kernel(obj_label, qus_type, attention, score_matrix):
    from concourse.bass_utils import run_bass_kernel_spmd

    obj_label = np.asarray(obj_label)
    qus_type = np.asarray(qus_type)
    attention = np.asarray(attention, np.float32)
    score_matrix = np.asarray(score_matrix, np.float32)

    in_maps, core_rows = _route(obj_label, qus_type, attention, score_matrix)
    nc = build_nc(NSEC)
    nc.compile()
    res = run_bass_kernel_spmd(nc, in_maps, core_ids=list(range(NCORES)))

    out2d = np.empty((NUM_QT * PAIR, ROWLEN), np.float32)
    for core in range(NCORES):
        rows = core_rows[core]
        mask = rows >= 0
        out2d[rows[mask]] = res.results[core]["score_out"][mask]
    return out2d.reshape(NUM_QT, PAIR, OT, OT)
